# revision 7
# baseline (speedup 1.0000x reference)
"""Multi-head attention Trainium2 kernel (B=8, S=2048, EMB=768, H=4, Dh=192).

Strategy: data-parallel over batch - one batch element per NeuronCore, no
collectives. Everything SBUF-resident; the attention pipeline processes one
head x two 512-wide q-halves per step. Projections and the output projection
pair matmuls per weight slice. V/out biases are folded exactly on the host
(softmax rows sum to 1 => out += bo + Wo@bv).

vs the previous revision:
  - scores K=192 contraction: the 64-row remainder segment of each head is
    stored TWICE (partitions 0-63 and 64-127 of a dedicated tile), so the
    qx0/qx1 remainder matmuls run CONCURRENTLY as row-tiled 64-row matmuls
    (tile_position rows 0 and 64, different PSUM banks). Scores drop from
    4 to 3 array-passes per kt: 109us -> 82us of PE streaming.
  - startup DMA order: wk+bk+xk first (K-proj starts ~2us in), wo/ones
    deferred; ones columns via memset (onesd DRAM input dropped).
  - output DMA'd as fp16 (upcast on host): halves tail DMA traffic.
  - softmax exp: both 512-wide q-halves of a kt accumulate into ONE 2-bank
    PSUM tile and are exp'd by a single N=1024 ACTIVATE. The ACT instruction
    carries a ~352-cycle fixed overhead, and the exp stream was the measured
    phase-2 pacer on HW (halving exp work saved ~80us); merging halves the
    instruction+semaphore count: ~90us on HW.

Layouts (feature-on-partition everywhere except V):
  P1: QT[do,s] = Wq^T.T @ q^T  -> qt full tiles {0,2,3,5} + 4 dup-remainder
      tiles (64 rows duplicated across both partition halves); same KT.
      V[s,do] natural          -> v_sb  [128, 16, 4*(192+1)] (+ones col/head)
  P2: per block (qb, h):
        scoresT[k,q] = Kh^T.T @ Qh^T   (k on partitions)
          full 128-row seg: 2 sequential matmuls (qx0, qx1)
          64-row remainder: 2 concurrent row-tiled matmuls
        E = exp(scoresT*scale)          (ACT)
        outT[dh,q] = Vh.T @ E ; Z[q] = ones.T @ E  (rides V's ones column)
        outT *= 1/Z (gpsimd partition-broadcast of 1/Z; normalize on DVE)
  P3: out[s,e] = Oc^T.T @ Wo^T, per q-block right after its 4 heads finish.
"""

import sys

sys.path.insert(0, "/opt/trn_rl_repo")

import numpy as np

import concourse.bass as bass  # noqa: F401  (import keeps bass registered)
import concourse.mybir as mybir
import concourse.tile as tile
from concourse import bacc

B, S, EMB, HEADS = 8, 2048, 768, 4
DH = EMB // HEADS  # 192
NCORES = 8
P = 128
DI_TILES = EMB // P  # 6
S_TILES = S // P  # 16
QBLK = 512
N_QBLK = S // QBLK  # 4
EBLK = 384
SCALE = 1.0 / float(np.sqrt(DH))
VW = DH + 1  # 193 cols per head in the V tile (192 dh + ones)

F32 = mybir.dt.float32
MMDT = mybir.dt.float16  # matmul operand dtype (psum accumulation is fp32)

# full 128-row segment tile index per head, and remainder dim ranges:
# h0: dims 0..191   = full t0  + rem (128..191  = t1[0:64])
# h1: dims 192..383 = rem (192..255 = t1[64:128]) + full t2
# h2: dims 384..575 = full t3  + rem (512..575  = t4[0:64])
# h3: dims 576..767 = rem (576..639 = t4[64:128]) + full t5
FULL_TILE = {0: 0, 1: 2, 2: 3, 3: 5}
REM_FIRST = {0: False, 1: True, 2: False, 3: True}  # rem before full?
# rem tile r holds its head's 64 dims on BOTH partition halves; the half
# written directly by the projection DVE (matching psum partitions):
REM_NATURAL_LO = {0: True, 1: False, 2: True, 3: False}


def _np_mmdt():
    return np.float16


def _build_nc(reps=1, phases=3):
    nc = bacc.Bacc("TRN2", target_bir_lowering=False, debug=False,
                   num_devices=NCORES)

    xq = nc.declare_dram_parameter("xq", [EMB, S], MMDT, isOutput=False)
    xk = nc.declare_dram_parameter("xk", [EMB, S], MMDT, isOutput=False)
    xv = nc.declare_dram_parameter("xv", [EMB, S], MMDT, isOutput=False)
    wq = nc.declare_dram_parameter("wq", [EMB, EMB], MMDT, isOutput=False)
    wk = nc.declare_dram_parameter("wk", [EMB, EMB], MMDT, isOutput=False)
    wv = nc.declare_dram_parameter("wv", [EMB, EMB], MMDT, isOutput=False)
    wo = nc.declare_dram_parameter("wo", [EMB, EMB], MMDT, isOutput=False)
    bq = nc.declare_dram_parameter("bq", [EMB, 1], F32, isOutput=False)
    bk = nc.declare_dram_parameter("bk", [EMB, 1], F32, isOutput=False)
    out = nc.declare_dram_parameter("out", [S, EMB], MMDT, isOutput=True)

    with tile.TileContext(nc) as tc:
        with tc.tile_pool(name="res", bufs=1) as res:
            # ---- persistent SBUF residents ----
            kt_sb = {j: res.tile([P, S], MMDT, name=f"kt{j}", tag=f"kt{j}")
                     for j in FULL_TILE.values()}
            qt_sb = {j: res.tile([P, S], MMDT, name=f"qt{j}", tag=f"qt{j}")
                     for j in FULL_TILE.values()}
            ktr = [res.tile([P, S], MMDT, name=f"ktr{h}", tag=f"ktr{h}")
                   for h in range(HEADS)]
            qtr = [res.tile([P, S], MMDT, name=f"qtr{h}", tag=f"qtr{h}")
                   for h in range(HEADS)]
            v_sb = res.tile([P, S_TILES, HEADS * VW], MMDT, name="v_sb")
            wo_t = [res.tile([P, EMB], MMDT, name=f"wo{i}", tag=f"wo{i}")
                    for i in range(DI_TILES)]

            # all 4 heads' ones columns in V, via memset (no DMA traffic)
            nc.vector.memset(
                v_sb.rearrange("p t (h c) -> p t h c", c=VW)[:, :, :, DH], 1.0)

            def proj_store(pss, bt, full_dst, rem_dst, do):
                """DVE bias-add psum->SBUF for one do-tile (4 q-blocks)."""
                with nc.allow_low_precision(
                        reason="fp16 storage of projections"):
                    for sb in range(N_QBLK):
                        cols = slice(sb * QBLK, (sb + 1) * QBLK)
                        if do in (1, 4):
                            ra, rb = rem_dst  # (lo-half head, hi-half head)
                            nc.vector.tensor_scalar_add(
                                ra[0:64, cols], pss[sb][0:64, :], bt[0:64])
                            nc.vector.tensor_scalar_add(
                                rb[64:P, cols], pss[sb][64:P, :], bt[64:P])
                        else:
                            nc.vector.tensor_scalar_add(
                                full_dst[:, cols], pss[sb], bt)

            for rep in range(reps):
                # ============ Phase 1: projections (K, V, Q) ============
                with tc.tile_pool(name=f"w1_{rep}", bufs=1) as wp, \
                     tc.tile_pool(name=f"x1_{rep}", bufs=2) as xp, \
                     tc.tile_pool(name=f"ps1_{rep}", bufs=8,
                                  space="PSUM") as psgen:
                    wk_t = [wp.tile([P, EMB], MMDT, name=f"wk{i}", tag=f"wk{i}")
                            for i in range(DI_TILES)]
                    wq_t = [wp.tile([P, EMB], MMDT, name=f"wq{i}", tag=f"wq{i}")
                            for i in range(DI_TILES)]
                    wv_t = [wp.tile([P, EMB], MMDT, name=f"wv{i}", tag=f"wv{i}")
                            for i in range(DI_TILES)]
                    bq_t = [wp.tile([P, 1], F32, name=f"bq{i}", tag=f"bq{i}")
                            for i in range(DI_TILES)]
                    bk_t = [wp.tile([P, 1], F32, name=f"bk{i}", tag=f"bk{i}")
                            for i in range(DI_TILES)]
                    # DMA issue order = ring order, sorted by compute need:
                    # wk/xk interleaved (first K matmul ~2us in), biases and
                    # everything else after.
                    xs_k, xs_q = [], []
                    for j in range(DI_TILES):
                        nc.sync.dma_start(out=wk_t[j], in_=wk[j * P:(j + 1) * P, :])
                        t = xp.tile([P, S], MMDT, name=f"xr{j}", tag=f"xr{j}")
                        nc.sync.dma_start(out=t, in_=xk[j * P:(j + 1) * P, :])
                        xs_k.append(t)
                    for i in range(DI_TILES):
                        nc.sync.dma_start(out=bk_t[i], in_=bk[i * P:(i + 1) * P, :])
                    for i in range(DI_TILES):
                        nc.sync.dma_start(out=wv_t[i], in_=wv[i * P:(i + 1) * P, :])
                    for i in range(DI_TILES):
                        nc.sync.dma_start(out=wq_t[i], in_=wq[i * P:(i + 1) * P, :])
                    for j in range(DI_TILES):
                        t = xp.tile([P, S], MMDT, name=f"xq{j}", tag=f"xr{j}")
                        nc.sync.dma_start(out=t, in_=xq[j * P:(j + 1) * P, :])
                        xs_q.append(t)
                    for i in range(DI_TILES):
                        nc.sync.dma_start(out=bq_t[i], in_=bq[i * P:(i + 1) * P, :])

                    # K then Q: transposed projections into resident tiles.
                    for (which, wt, bt, full_map, rem_list, xs) in (
                            ("k", wk_t, bk_t, kt_sb, ktr, xs_k),
                            ("q", wq_t, bq_t, qt_sb, qtr, xs_q)):
                        for do in range(DI_TILES):
                            # 4-way stationary reuse: one weight-slice load
                            # feeds all four 512-wide s-blocks
                            pss = [psgen.tile([P, QBLK], F32, name="gen",
                                              tag="gen")
                                   for _ in range(N_QBLK)]
                            for di in range(DI_TILES):
                                wsl = wt[di][:, do * P:(do + 1) * P]
                                for sb in range(N_QBLK):
                                    nc.tensor.matmul(
                                        pss[sb], wsl,
                                        xs[di][:, sb * QBLK:(sb + 1) * QBLK],
                                        start=(di == 0),
                                        stop=(di == DI_TILES - 1))
                            if do == 1:
                                proj_store(pss, bt[do], None,
                                           (rem_list[0], rem_list[1]), do)
                            elif do == 4:
                                proj_store(pss, bt[do], None,
                                           (rem_list[2], rem_list[3]), do)
                            else:
                                proj_store(pss, bt[do], full_map[do], None, do)
                        if which == "k":
                            # V projection (natural layout, per-head ones col)
                            for sb in range(N_QBLK):
                                scols = slice(sb * QBLK, (sb + 1) * QBLK)
                                vs = []
                                for j in range(DI_TILES):
                                    t = xp.tile([P, QBLK], MMDT, name=f"vs{j}",
                                                tag=f"vs{j}")
                                    nc.sync.dma_start(
                                        out=t, in_=xv[j * P:(j + 1) * P, scols])
                                    vs.append(t)
                                for sti in range(QBLK // P):
                                    st = sb * (QBLK // P) + sti
                                    pcols = slice(sti * P, (sti + 1) * P)
                                    # di-outer: both head-halves' matmuls
                                    # share the stationary x slice
                                    pss = [psgen.tile([P, QBLK], F32,
                                                      name="gen", tag="gen")
                                           for _ in range(2)]
                                    for di in range(DI_TILES):
                                        for blk in range(2):  # heads {0,1}/{2,3}
                                            nc.tensor.matmul(
                                                pss[blk][:, 0:EBLK],
                                                vs[di][:, pcols],
                                                wv_t[di][:, blk * EBLK:(blk + 1) * EBLK],
                                                start=(di == 0),
                                                stop=(di == DI_TILES - 1))
                                    for blk in range(2):
                                        dst = v_sb[:, st,
                                                   blk * 2 * VW:(blk * 2 + 2) * VW]
                                        dst = dst.rearrange(
                                            "p (h c) -> p h c", c=VW)[:, :, 0:DH]
                                        with nc.allow_low_precision(
                                                reason="fp16 storage of V"):
                                            nc.vector.tensor_copy(
                                                dst,
                                                pss[blk][:, 0:EBLK].rearrange(
                                                    "p (h c) -> p h c", c=DH))
                        # duplicate each head's remainder rows onto the other
                        # partition half (SBUF->SBUF DMA; off critical path,
                        # emitted after the xv stream so it doesn't delay it)
                        for h in range(HEADS):
                            r = rem_list[h]
                            if REM_NATURAL_LO[h]:
                                nc.gpsimd.dma_start(out=r[64:P, :],
                                                    in_=r[0:64, :])
                            else:
                                nc.gpsimd.dma_start(out=r[0:64, :],
                                                    in_=r[64:P, :])
                    if rep == 0:
                        for i in range(DI_TILES):
                            nc.sync.dma_start(out=wo_t[i],
                                              in_=wo[i * P:(i + 1) * P, :])

                if phases < 2:
                    continue

                # ============ Phase 2+3: pipelined attention ============
                with tc.tile_pool(name=f"ee_{rep}", bufs=2) as eep, \
                     tc.tile_pool(name=f"ps2_{rep}", bufs=2,
                                  space="PSUM") as ps2, \
                     tc.tile_pool(name=f"zz_{rep}", bufs=2) as zzp, \
                     tc.tile_pool(name=f"fe_{rep}", bufs=4) as fep, \
                     tc.tile_pool(name=f"oc_{rep}", bufs=2) as ocp, \
                     tc.tile_pool(name=f"pso1_{rep}", bufs=2, space="PSUM") as pso1, \
                     tc.tile_pool(name=f"pso2_{rep}", bufs=2, space="PSUM") as pso2:

                    blocks = [(qbp, h) for qbp in range(N_QBLK // 2)
                              for h in range(HEADS)]
                    nb = len(blocks)
                    st_e = {}   # step -> (e_q0, e_q1)
                    st_o = {}   # step -> ((o1, o2) for q0, (o1, o2) for q1)
                    st_rz = {}  # step -> (rz_q0, rz_q1)
                    oc_cur = {}  # qb -> [6 oc tiles]

                    def oc_tiles(qb):
                        if qb not in oc_cur:
                            oc_cur[qb] = [ocp.tile([P, QBLK], MMDT,
                                                   name=f"oc{j}", tag=f"oc{j}")
                                          for j in range(DI_TILES)]
                        return oc_cur[qb]

                    def scores_prep(s):
                        st_e[s] = eep.tile([P, S_TILES, 2 * QBLK], MMDT,
                                           name="E", tag="E")

                    def scores_kt(s, kt):
                        qbp, h = blocks[s]
                        kc = slice(kt * P, (kt + 1) * P)
                        qc = [slice((2 * qbp + qx) * QBLK,
                                    (2 * qbp + qx + 1) * QBLK)
                              for qx in range(2)]
                        # one fat 2-bank psum per kt: qx0 in bank 0, qx1 in
                        # bank 1, so ONE N=1024 ACT exp serves both q-halves
                        # (the ACTIVATE has a 352-cycle fixed overhead; at
                        # N=512 that is ~40% of the instruction)
                        pse = ps2.tile([P, 2, QBLK], F32, name="gen2",
                                       tag="gen2")
                        rem_first = REM_FIRST[h]

                        def emit_full(start, stop):
                            j = FULL_TILE[h]
                            for qx in range(2):
                                nc.tensor.matmul(
                                    pse[:, qx, :], kt_sb[j][:, kc],
                                    qt_sb[j][:, qc[qx]],
                                    start=start, stop=stop)

                        def emit_rem(start, stop):
                            # concurrent row-tiled pair: qx0 on array rows
                            # 0-63, qx1 on rows 64-127 (different PSUM banks)
                            nc.tensor.matmul(
                                pse[:, 0, :], ktr[h][0:64, kc],
                                qtr[h][0:64, qc[0]],
                                start=start, stop=stop)
                            nc.tensor.matmul(
                                pse[:, 1, :], ktr[h][64:P, kc],
                                qtr[h][64:P, qc[1]],
                                start=start, stop=stop)

                        if rem_first:
                            emit_rem(True, False)
                            emit_full(False, True)
                        else:
                            emit_full(True, False)
                            emit_rem(False, True)
                        nc.scalar.activation(
                            st_e[s][:, kt, :],
                            pse.rearrange("p a b -> p (a b)"),
                            mybir.ActivationFunctionType.Exp,
                            bias=0.0, scale=SCALE)

                    def attv_prep(s):
                        st_o[s] = tuple(
                            (pso1.tile([P, QBLK], F32, name="o1", tag="o1"),
                             pso2.tile([DH + 1 - P, QBLK], F32, name="o2",
                                       tag="o2"))
                            for _ in range(2))

                    def attv_kt(s, kt):
                        qbp, h = blocks[s]
                        last = kt == S_TILES - 1
                        for part in range(2):
                            vsl = (v_sb[:, kt, h * VW:h * VW + P] if part == 0
                                   else v_sb[:, kt, h * VW + P:(h + 1) * VW])
                            for qx in range(2):
                                nc.tensor.matmul(
                                    st_o[s][qx][part], vsl,
                                    st_e[s][:, kt,
                                            qx * QBLK:(qx + 1) * QBLK],
                                    start=(kt == 0), stop=last)

                    def attv_fin(s):
                        rzs = []
                        for qx in range(2):
                            rz = zzp.tile([1, QBLK], MMDT, name="rz", tag="rz")
                            with nc.allow_low_precision(
                                    reason="softmax reciprocal, fp16 storage"):
                                nc.vector.reciprocal(
                                    rz, st_o[s][qx][1][DH - P:DH - P + 1, :])
                            rzs.append(rz)
                        st_rz[s] = tuple(rzs)

                    def emit_norm(s):
                        qbp, h = blocks[s]
                        opair = st_o.pop(s)
                        rzs = st_rz.pop(s)
                        del st_e[s]
                        segs = sorted({0, DH, P} |
                                      {j * P - h * DH for j in range(DI_TILES + 1)
                                       if 0 < j * P - h * DH < DH})
                        for qx in range(2):
                            ps_o1, ps_o2 = opair[qx]
                            # broadcast 1/z across partitions on the (idle)
                            # gpsimd engine - no PE matmul, no DVE copy
                            bz = zzp.tile([P, QBLK], MMDT, name="bz", tag="bz")
                            nc.gpsimd.partition_broadcast(bz, rzs[qx][0:1, :])
                            oc = oc_tiles(2 * qbp + qx)
                            with nc.allow_low_precision(
                                    reason="softmax normalize, fp16 storage"):
                                for a, b in zip(segs[:-1], segs[1:]):
                                    r = h * DH + a
                                    j, p0 = divmod(r, P)
                                    src = (ps_o1[a:b, :] if b <= P
                                           else ps_o2[a - P:b - P, :])
                                    nc.vector.tensor_mul(
                                        oc[j][p0:p0 + (b - a), :],
                                        src, bz[0:b - a, :])

                    def emit_phase3(qb):
                        oc = oc_cur.pop(qb)
                        if phases < 3:
                            return
                        for sti in range(QBLK // P):
                            st = qb * (QBLK // P) + sti
                            scols = slice(st * P, (st + 1) * P)
                            pcols = slice(sti * P, (sti + 1) * P)
                            # j-outer: both EBLK halves share the stationary
                            # oc slice; one fat tile holds both eb psums
                            pssf = ps2.tile([P, 2, QBLK], F32, name="gen2",
                                            tag="gen2")
                            for j in range(DI_TILES):
                                for eb in range(2):
                                    nc.tensor.matmul(
                                        pssf[:, eb, 0:EBLK], oc[j][:, pcols],
                                        wo_t[j][:, eb * EBLK:(eb + 1) * EBLK],
                                        start=(j == 0),
                                        stop=(j == DI_TILES - 1))
                            fin = fep.tile([P, EMB], MMDT, name="fin",
                                           tag="fin")
                            with nc.allow_low_precision(
                                    reason="fp16 output storage"):
                                nc.vector.tensor_copy(
                                    fin.rearrange("p (a b) -> p a b", b=EBLK),
                                    pssf[:, :, 0:EBLK])
                            nc.gpsimd.dma_start(out=out[scols, :], in_=fin)

                    # software pipeline over steps, merged at kt granularity
                    LAG = 6
                    pending_p3 = []
                    scores_prep(0)
                    for kt in range(S_TILES):
                        scores_kt(0, kt)
                    for s in range(nb):
                        has_next = s + 1 < nb
                        if has_next:
                            scores_prep(s + 1)
                        attv_prep(s)
                        nkt = S_TILES + LAG if has_next else S_TILES
                        for kt in range(nkt):
                            if has_next and kt < S_TILES:
                                scores_kt(s + 1, kt)
                            if kt == 0 and s - 1 >= 0:
                                emit_norm(s - 1)
                                qbp_prev, h_prev = blocks[s - 1]
                                if h_prev == HEADS - 1:
                                    pending_p3 += [2 * qbp_prev,
                                                   2 * qbp_prev + 1]
                            akt = kt - LAG if has_next else kt
                            if 0 <= akt < S_TILES:
                                attv_kt(s, akt)
                        attv_fin(s)
                        for qb in pending_p3:
                            emit_phase3(qb)
                        pending_p3 = []
                    emit_norm(nb - 1)
                    emit_phase3(N_QBLK - 2)
                    emit_phase3(N_QBLK - 1)

    nc.compile()
    return nc


_CACHE = {}


def _get_runner(reps=1, phases=3):
    """Build nc once and a reusable jitted SPMD callable (no recompiles)."""
    key = f"runner{reps}_{phases}"
    if key in _CACHE:
        return _CACHE[key]

    import jax
    import numpy as _np
    from jax.sharding import Mesh, PartitionSpec
    from jax.experimental.shard_map import shard_map
    from concourse import bass2jax
    from concourse.bass2jax import _bass_exec_p, install_neuronx_cc_hook

    nc = _build_nc(reps, phases)
    install_neuronx_cc_hook()

    partition_name = (nc.partition_id_tensor.name
                      if nc.partition_id_tensor else None)
    in_names, out_names, out_avals, zero_outs = [], [], [], []
    for alloc in nc.m.functions[0].allocations:
        if not isinstance(alloc, mybir.MemoryLocationSet):
            continue
        name = alloc.memorylocations[0].name
        if alloc.kind == "ExternalInput":
            if name != partition_name:
                in_names.append(name)
        elif alloc.kind == "ExternalOutput":
            shape = list(alloc.tensor_shape)
            npdt = mybir.dt.np(alloc.dtype)
            out_avals.append(jax.core.ShapedArray(shape, npdt))
            out_names.append(name)
            zero_outs.append(_np.zeros(shape, npdt))
    n_params = len(in_names)
    n_outs = len(out_names)
    in_names = in_names + out_names
    if partition_name is not None:
        in_names.append(partition_name)

    def _body(*args):
        operands = list(args)
        if partition_name is not None:
            operands.append(bass2jax.partition_id_tensor())
        outs = _bass_exec_p.bind(
            *operands,
            out_avals=tuple(out_avals),
            in_names=tuple(in_names),
            out_names=tuple(out_names),
            lowering_input_output_aliases=(),
            sim_require_finite=True,
            sim_require_nnan=True,
            nc=nc,
        )
        return tuple(outs)

    devices = jax.devices()[:NCORES]
    mesh = Mesh(_np.asarray(devices), ("core",))
    in_specs = (PartitionSpec("core"),) * (n_params + n_outs)
    out_specs = (PartitionSpec("core"),) * n_outs
    sharded = jax.jit(
        shard_map(_body, mesh=mesh, in_specs=in_specs, out_specs=out_specs,
                  check_rep=False),
        keep_unused=True,
    )
    concat_zeros = [
        _np.zeros((NCORES * z.shape[0], *z.shape[1:]), z.dtype)
        for z in zero_outs
    ]

    runner = {
        "nc": nc, "sharded": sharded, "in_names": in_names,
        "n_params": n_params, "out_names": out_names,
        "out_avals": out_avals, "concat_zeros": concat_zeros,
        "mesh": mesh,
    }
    _CACHE[key] = runner
    return runner


def run_spmd(in_maps):
    """Run the compiled SPMD program; in_maps is a list of NCORES dicts."""
    import numpy as _np
    r = _get_runner()
    per_core = [[_np.asarray(m[name]) for name in r["in_names"][:r["n_params"]]]
                for m in in_maps]
    concat_in = [
        _np.concatenate([per_core[c][i] for c in range(NCORES)], axis=0)
        for i in range(r["n_params"])
    ]
    out_arrs = r["sharded"](*concat_in, *r["concat_zeros"])
    return [
        {name: _np.asarray(out_arrs[i]).reshape(NCORES, *r["out_avals"][i].shape)[c]
         for i, name in enumerate(r["out_names"])}
        for c in range(NCORES)
    ]


def _prep_in_maps(q, k, v, Wq, bq, Wk, bk, Wv, bv, Wo, bo):
    mdt = _np_mmdt()
    q = np.asarray(q, dtype=np.float32)
    k = np.asarray(k, dtype=np.float32)
    v = np.asarray(v, dtype=np.float32)
    wqT = np.ascontiguousarray(np.asarray(Wq, np.float32).T).astype(mdt)
    wkT = np.ascontiguousarray(np.asarray(Wk, np.float32).T).astype(mdt)
    wvT = np.ascontiguousarray(np.asarray(Wv, np.float32).T).astype(mdt)
    woT = np.ascontiguousarray(np.asarray(Wo, np.float32).T).astype(mdt)
    bqc = np.ascontiguousarray(np.asarray(bq, np.float32).reshape(EMB, 1))
    bkc = np.ascontiguousarray(np.asarray(bk, np.float32).reshape(EMB, 1))
    in_maps = []
    for b in range(NCORES):
        in_maps.append({
            "xq": np.ascontiguousarray(q[b].T).astype(mdt),
            "xk": np.ascontiguousarray(k[b].T).astype(mdt),
            "xv": np.ascontiguousarray(v[b].T).astype(mdt),
            "wq": wqT, "wk": wkT, "wv": wvT, "wo": woT,
            "bq": bqc, "bk": bkc,
        })
    return in_maps


def kernel(q, k, v, Wq, bq, Wk, bk, Wv, bv, Wo, bo):
    in_maps = _prep_in_maps(q, k, v, Wq, bq, Wk, bk, Wv, bv, Wo, bo)
    results = run_spmd(in_maps)
    out = np.stack([results[b]["out"] for b in range(NCORES)], axis=0)
    out = out.astype(np.float32)
    # exact epilogue: softmax rows sum to 1, so the V bias contributes
    # bv @ Wo.T to every output row; fold it with bo on the host.
    extra = (np.asarray(bo, np.float32)
             + np.asarray(Wo, np.float32) @ np.asarray(bv, np.float32))
    if np.any(extra):
        out = out + extra
    return out


# revision 8
# speedup vs baseline: 1.2197x; 1.2197x over previous
"""Multi-head attention Trainium2 kernel (B=8, S=2048, EMB=768, H=4, Dh=192).

Strategy: data-parallel over batch - one batch element per NeuronCore, no
collectives. Everything SBUF-resident; the attention pipeline processes one
head x two 512-wide q-halves per step. Projections and the output projection
pair matmuls per weight slice. V/out biases are folded exactly on the host
(softmax rows sum to 1 => out += bo + Wo@bv).

vs the previous revision:
  - scores K=192 contraction: the 64-row remainder segment of each head is
    stored TWICE (partitions 0-63 and 64-127 of a dedicated tile), so the
    qx0/qx1 remainder matmuls run CONCURRENTLY as row-tiled 64-row matmuls
    (tile_position rows 0 and 64, different PSUM banks). Scores drop from
    4 to 3 array-passes per kt: 109us -> 82us of PE streaming.
  - startup DMA order: wk+bk+xk first (K-proj starts ~2us in), wo/ones
    deferred; ones columns via memset (onesd DRAM input dropped).
  - output DMA'd as fp16 (upcast on host): halves tail DMA traffic.
  - softmax exp: both 512-wide q-halves of a kt accumulate into ONE 2-bank
    PSUM tile and are exp'd by a single N=1024 ACTIVATE. The ACT instruction
    carries a ~352-cycle fixed overhead, and the exp stream was the measured
    phase-2 pacer on HW (halving exp work saved ~80us); merging halves the
    instruction+semaphore count: ~90us on HW.

Layouts (feature-on-partition everywhere except V):
  P1: QT[do,s] = Wq^T.T @ q^T  -> qt full tiles {0,2,3,5} + 4 dup-remainder
      tiles (64 rows duplicated across both partition halves); same KT.
      V[s,do] natural          -> v_sb  [128, 16, 4*(192+1)] (+ones col/head)
  P2: per block (qb, h):
        scoresT[k,q] = Kh^T.T @ Qh^T   (k on partitions)
          full 128-row seg: 2 sequential matmuls (qx0, qx1)
          64-row remainder: 2 concurrent row-tiled matmuls
        E = exp(scoresT*scale)          (ACT)
        outT[dh,q] = Vh.T @ E ; Z[q] = ones.T @ E  (rides V's ones column)
        outT *= 1/Z (gpsimd partition-broadcast of 1/Z; normalize on DVE)
  P3: out[s,e] = Oc^T.T @ Wo^T, per q-block right after its 4 heads finish.
"""

import sys

sys.path.insert(0, "/opt/trn_rl_repo")

import numpy as np

import concourse.bass as bass  # noqa: F401  (import keeps bass registered)
import concourse.mybir as mybir
import concourse.tile as tile
from concourse import bacc

B, S, EMB, HEADS = 8, 2048, 768, 4
DH = EMB // HEADS  # 192
NCORES = 8
P = 128
DI_TILES = EMB // P  # 6
S_TILES = S // P  # 16
QBLK = 512
N_QBLK = S // QBLK  # 4
EBLK = 384
SCALE = 1.0 / float(np.sqrt(DH))
VW = DH + 1  # 193 cols per head in the V tile (192 dh + ones)

F32 = mybir.dt.float32
MMDT = mybir.dt.float16  # matmul operand dtype (psum accumulation is fp32)

# full 128-row segment tile index per head, and remainder dim ranges:
# h0: dims 0..191   = full t0  + rem (128..191  = t1[0:64])
# h1: dims 192..383 = rem (192..255 = t1[64:128]) + full t2
# h2: dims 384..575 = full t3  + rem (512..575  = t4[0:64])
# h3: dims 576..767 = rem (576..639 = t4[64:128]) + full t5
FULL_TILE = {0: 0, 1: 2, 2: 3, 3: 5}
REM_FIRST = {0: False, 1: True, 2: False, 3: True}  # rem before full?
# rem tile r holds its head's 64 dims on BOTH partition halves; the half
# written directly by the projection DVE (matching psum partitions):
REM_NATURAL_LO = {0: True, 1: False, 2: True, 3: False}


def _np_mmdt():
    return np.float16


def _build_nc(reps=1, phases=3):
    nc = bacc.Bacc("TRN2", target_bir_lowering=False, debug=False,
                   num_devices=NCORES)

    xq = nc.declare_dram_parameter("xq", [EMB, S], MMDT, isOutput=False)
    xk = nc.declare_dram_parameter("xk", [EMB, S], MMDT, isOutput=False)
    xv = nc.declare_dram_parameter("xv", [EMB, S], MMDT, isOutput=False)
    wq = nc.declare_dram_parameter("wq", [EMB, EMB], MMDT, isOutput=False)
    wk = nc.declare_dram_parameter("wk", [EMB, EMB], MMDT, isOutput=False)
    wv = nc.declare_dram_parameter("wv", [EMB, EMB], MMDT, isOutput=False)
    wo = nc.declare_dram_parameter("wo", [EMB, EMB], MMDT, isOutput=False)
    bq = nc.declare_dram_parameter("bq", [EMB, 1], F32, isOutput=False)
    bk = nc.declare_dram_parameter("bk", [EMB, 1], F32, isOutput=False)
    out = nc.declare_dram_parameter("out", [S, EMB], MMDT, isOutput=True)

    with tile.TileContext(nc) as tc:
        with tc.tile_pool(name="res", bufs=1) as res:
            # ---- persistent SBUF residents ----
            kt_sb = {j: res.tile([P, S], MMDT, name=f"kt{j}", tag=f"kt{j}")
                     for j in FULL_TILE.values()}
            qt_sb = {j: res.tile([P, S], MMDT, name=f"qt{j}", tag=f"qt{j}")
                     for j in FULL_TILE.values()}
            ktr = [res.tile([P, S], MMDT, name=f"ktr{h}", tag=f"ktr{h}")
                   for h in range(HEADS)]
            qtr = [res.tile([P, S], MMDT, name=f"qtr{h}", tag=f"qtr{h}")
                   for h in range(HEADS)]
            v_sb = res.tile([P, S_TILES, HEADS * VW], MMDT, name="v_sb")
            wo_t = [res.tile([P, EMB], MMDT, name=f"wo{i}", tag=f"wo{i}")
                    for i in range(DI_TILES)]

            # all 4 heads' ones columns in V, via memset (no DMA traffic)
            nc.vector.memset(
                v_sb.rearrange("p t (h c) -> p t h c", c=VW)[:, :, :, DH], 1.0)

            def proj_store(pss, bt, full_dst, rem_dst, do):
                """DVE bias-add psum->SBUF for one do-tile (4 q-blocks)."""
                with nc.allow_low_precision(
                        reason="fp16 storage of projections"):
                    for sb in range(N_QBLK):
                        cols = slice(sb * QBLK, (sb + 1) * QBLK)
                        if do in (1, 4):
                            ra, rb = rem_dst  # (lo-half head, hi-half head)
                            nc.vector.tensor_scalar_add(
                                ra[0:64, cols], pss[sb][0:64, :], bt[0:64])
                            nc.vector.tensor_scalar_add(
                                rb[64:P, cols], pss[sb][64:P, :], bt[64:P])
                        else:
                            nc.vector.tensor_scalar_add(
                                full_dst[:, cols], pss[sb], bt)

            for rep in range(reps):
                # ============ Phase 1: projections (K, V, Q) ============
                with tc.tile_pool(name=f"w1_{rep}", bufs=1) as wp, \
                     tc.tile_pool(name=f"x1_{rep}", bufs=2) as xp, \
                     tc.tile_pool(name=f"ps1_{rep}", bufs=8,
                                  space="PSUM") as psgen:
                    wk_t = [wp.tile([P, EMB], MMDT, name=f"wk{i}", tag=f"wk{i}")
                            for i in range(DI_TILES)]
                    wq_t = [wp.tile([P, EMB], MMDT, name=f"wq{i}", tag=f"wq{i}")
                            for i in range(DI_TILES)]
                    wv_t = [wp.tile([P, EMB], MMDT, name=f"wv{i}", tag=f"wv{i}")
                            for i in range(DI_TILES)]
                    bq_t = [wp.tile([P, 1], F32, name=f"bq{i}", tag=f"bq{i}")
                            for i in range(DI_TILES)]
                    bk_t = [wp.tile([P, 1], F32, name=f"bk{i}", tag=f"bk{i}")
                            for i in range(DI_TILES)]
                    # DMA issue order = ring order, sorted by compute need:
                    # wk/xk interleaved (first K matmul ~2us in), biases and
                    # everything else after.
                    xs_k, xs_q = [], []
                    for j in range(DI_TILES):
                        nc.sync.dma_start(out=wk_t[j], in_=wk[j * P:(j + 1) * P, :])
                        t = xp.tile([P, S], MMDT, name=f"xr{j}", tag=f"xr{j}")
                        nc.sync.dma_start(out=t, in_=xk[j * P:(j + 1) * P, :])
                        xs_k.append(t)
                    for i in range(DI_TILES):
                        nc.sync.dma_start(out=bk_t[i], in_=bk[i * P:(i + 1) * P, :])
                    for i in range(DI_TILES):
                        nc.sync.dma_start(out=wv_t[i], in_=wv[i * P:(i + 1) * P, :])
                    for i in range(DI_TILES):
                        nc.sync.dma_start(out=wq_t[i], in_=wq[i * P:(i + 1) * P, :])
                    for j in range(DI_TILES):
                        t = xp.tile([P, S], MMDT, name=f"xq{j}", tag=f"xr{j}")
                        nc.sync.dma_start(out=t, in_=xq[j * P:(j + 1) * P, :])
                        xs_q.append(t)
                    for i in range(DI_TILES):
                        nc.sync.dma_start(out=bq_t[i], in_=bq[i * P:(i + 1) * P, :])

                    # K then Q: transposed projections into resident tiles.
                    for (which, wt, bt, full_map, rem_list, xs) in (
                            ("k", wk_t, bk_t, kt_sb, ktr, xs_k),
                            ("q", wq_t, bq_t, qt_sb, qtr, xs_q)):
                        for do in range(DI_TILES):
                            # 4-way stationary reuse: one weight-slice load
                            # feeds all four 512-wide s-blocks
                            pss = [psgen.tile([P, QBLK], F32, name="gen",
                                              tag="gen")
                                   for _ in range(N_QBLK)]
                            for di in range(DI_TILES):
                                wsl = wt[di][:, do * P:(do + 1) * P]
                                for sb in range(N_QBLK):
                                    nc.tensor.matmul(
                                        pss[sb], wsl,
                                        xs[di][:, sb * QBLK:(sb + 1) * QBLK],
                                        start=(di == 0),
                                        stop=(di == DI_TILES - 1))
                            if do == 1:
                                proj_store(pss, bt[do], None,
                                           (rem_list[0], rem_list[1]), do)
                            elif do == 4:
                                proj_store(pss, bt[do], None,
                                           (rem_list[2], rem_list[3]), do)
                            else:
                                proj_store(pss, bt[do], full_map[do], None, do)
                        if which == "k":
                            # V projection (natural layout, per-head ones col)
                            for sb in range(N_QBLK):
                                scols = slice(sb * QBLK, (sb + 1) * QBLK)
                                vs = []
                                for j in range(DI_TILES):
                                    t = xp.tile([P, QBLK], MMDT, name=f"vs{j}",
                                                tag=f"vs{j}")
                                    nc.sync.dma_start(
                                        out=t, in_=xv[j * P:(j + 1) * P, scols])
                                    vs.append(t)
                                for sti in range(QBLK // P):
                                    st = sb * (QBLK // P) + sti
                                    pcols = slice(sti * P, (sti + 1) * P)
                                    # di-outer: both head-halves' matmuls
                                    # share the stationary x slice
                                    pss = [psgen.tile([P, QBLK], F32,
                                                      name="gen", tag="gen")
                                           for _ in range(2)]
                                    for di in range(DI_TILES):
                                        for blk in range(2):  # heads {0,1}/{2,3}
                                            nc.tensor.matmul(
                                                pss[blk][:, 0:EBLK],
                                                vs[di][:, pcols],
                                                wv_t[di][:, blk * EBLK:(blk + 1) * EBLK],
                                                start=(di == 0),
                                                stop=(di == DI_TILES - 1))
                                    for blk in range(2):
                                        dst = v_sb[:, st,
                                                   blk * 2 * VW:(blk * 2 + 2) * VW]
                                        dst = dst.rearrange(
                                            "p (h c) -> p h c", c=VW)[:, :, 0:DH]
                                        with nc.allow_low_precision(
                                                reason="fp16 storage of V"):
                                            nc.vector.tensor_copy(
                                                dst,
                                                pss[blk][:, 0:EBLK].rearrange(
                                                    "p (h c) -> p h c", c=DH))
                        # duplicate each head's remainder rows onto the other
                        # partition half (SBUF->SBUF DMA; off critical path,
                        # emitted after the xv stream so it doesn't delay it)
                        for h in range(HEADS):
                            r = rem_list[h]
                            if REM_NATURAL_LO[h]:
                                nc.gpsimd.dma_start(out=r[64:P, :],
                                                    in_=r[0:64, :])
                            else:
                                nc.gpsimd.dma_start(out=r[0:64, :],
                                                    in_=r[64:P, :])
                    if rep == 0:
                        for i in range(DI_TILES):
                            nc.sync.dma_start(out=wo_t[i],
                                              in_=wo[i * P:(i + 1) * P, :])

                if phases < 2:
                    continue

                # ============ Phase 2+3: pipelined attention ============
                with tc.tile_pool(name=f"ee_{rep}", bufs=2) as eep, \
                     tc.tile_pool(name=f"ps2_{rep}", bufs=2,
                                  space="PSUM") as ps2, \
                     tc.tile_pool(name=f"zz_{rep}", bufs=2) as zzp, \
                     tc.tile_pool(name=f"fe_{rep}", bufs=4) as fep, \
                     tc.tile_pool(name=f"oc_{rep}", bufs=2) as ocp, \
                     tc.tile_pool(name=f"pso1_{rep}", bufs=2, space="PSUM") as pso1, \
                     tc.tile_pool(name=f"pso2_{rep}", bufs=2, space="PSUM") as pso2:

                    blocks = [(qbp, h) for qbp in range(N_QBLK // 2)
                              for h in range(HEADS)]
                    nb = len(blocks)
                    st_e = {}   # step -> (e_q0, e_q1)
                    st_o = {}   # step -> ((o1, o2) for q0, (o1, o2) for q1)
                    st_rz = {}  # step -> (rz_q0, rz_q1)
                    oc_cur = {}  # qb -> [6 oc tiles]

                    def oc_tiles(qb):
                        if qb not in oc_cur:
                            oc_cur[qb] = [ocp.tile([P, QBLK], MMDT,
                                                   name=f"oc{j}", tag=f"oc{j}")
                                          for j in range(DI_TILES)]
                        return oc_cur[qb]

                    def scores_prep(s):
                        st_e[s] = eep.tile([P, S_TILES, 2 * QBLK], MMDT,
                                           name="E", tag="E")

                    def scores_kt(s, kt):
                        qbp, h = blocks[s]
                        kc = slice(kt * P, (kt + 1) * P)
                        qc = [slice((2 * qbp + qx) * QBLK,
                                    (2 * qbp + qx + 1) * QBLK)
                              for qx in range(2)]
                        # one fat 2-bank psum per kt: qx0 in bank 0, qx1 in
                        # bank 1, so ONE N=1024 ACT exp serves both q-halves
                        # (the ACTIVATE has a 352-cycle fixed overhead; at
                        # N=512 that is ~40% of the instruction)
                        pse = ps2.tile([P, 2, QBLK], F32, name="gen2",
                                       tag="gen2")
                        rem_first = REM_FIRST[h]

                        def emit_full(start, stop):
                            j = FULL_TILE[h]
                            for qx in range(2):
                                nc.tensor.matmul(
                                    pse[:, qx, :], kt_sb[j][:, kc],
                                    qt_sb[j][:, qc[qx]],
                                    start=start, stop=stop)

                        def emit_rem(start, stop):
                            # concurrent row-tiled pair: qx0 on array rows
                            # 0-63, qx1 on rows 64-127 (different PSUM banks)
                            nc.tensor.matmul(
                                pse[:, 0, :], ktr[h][0:64, kc],
                                qtr[h][0:64, qc[0]],
                                start=start, stop=stop)
                            nc.tensor.matmul(
                                pse[:, 1, :], ktr[h][64:P, kc],
                                qtr[h][64:P, qc[1]],
                                start=start, stop=stop)

                        if rem_first:
                            emit_rem(True, False)
                            emit_full(False, True)
                        else:
                            emit_full(True, False)
                            emit_rem(False, True)
                        nc.scalar.activation(
                            st_e[s][:, kt, :],
                            pse.rearrange("p a b -> p (a b)"),
                            mybir.ActivationFunctionType.Exp,
                            bias=0.0, scale=SCALE)

                    def attv_prep(s):
                        st_o[s] = tuple(
                            (pso1.tile([P, QBLK], F32, name="o1", tag="o1"),
                             pso2.tile([DH + 1 - P, QBLK], F32, name="o2",
                                       tag="o2"))
                            for _ in range(2))

                    def attv_kt(s, kt):
                        qbp, h = blocks[s]
                        last = kt == S_TILES - 1
                        for part in range(2):
                            vsl = (v_sb[:, kt, h * VW:h * VW + P] if part == 0
                                   else v_sb[:, kt, h * VW + P:(h + 1) * VW])
                            for qx in range(2):
                                nc.tensor.matmul(
                                    st_o[s][qx][part], vsl,
                                    st_e[s][:, kt,
                                            qx * QBLK:(qx + 1) * QBLK],
                                    start=(kt == 0), stop=last)

                    def attv_fin(s):
                        rzs = []
                        for qx in range(2):
                            rz = zzp.tile([1, QBLK], MMDT, name="rz", tag="rz")
                            with nc.allow_low_precision(
                                    reason="softmax reciprocal, fp16 storage"):
                                nc.vector.reciprocal(
                                    rz, st_o[s][qx][1][DH - P:DH - P + 1, :])
                            rzs.append(rz)
                        st_rz[s] = tuple(rzs)

                    def emit_norm(s):
                        qbp, h = blocks[s]
                        opair = st_o.pop(s)
                        rzs = st_rz.pop(s)
                        del st_e[s]
                        segs = sorted({0, DH, P} |
                                      {j * P - h * DH for j in range(DI_TILES + 1)
                                       if 0 < j * P - h * DH < DH})
                        for qx in range(2):
                            ps_o1, ps_o2 = opair[qx]
                            # broadcast 1/z across partitions on the (idle)
                            # gpsimd engine - no PE matmul, no DVE copy
                            bz = zzp.tile([P, QBLK], MMDT, name="bz", tag="bz")
                            nc.gpsimd.partition_broadcast(bz, rzs[qx][0:1, :])
                            oc = oc_tiles(2 * qbp + qx)
                            with nc.allow_low_precision(
                                    reason="softmax normalize, fp16 storage"):
                                for a, b in zip(segs[:-1], segs[1:]):
                                    r = h * DH + a
                                    j, p0 = divmod(r, P)
                                    src = (ps_o1[a:b, :] if b <= P
                                           else ps_o2[a - P:b - P, :])
                                    nc.vector.tensor_mul(
                                        oc[j][p0:p0 + (b - a), :],
                                        src, bz[0:b - a, :])

                    def emit_phase3(qb):
                        oc = oc_cur.pop(qb)
                        if phases < 3:
                            return
                        for sti in range(QBLK // P):
                            st = qb * (QBLK // P) + sti
                            scols = slice(st * P, (st + 1) * P)
                            pcols = slice(sti * P, (sti + 1) * P)
                            # j-outer: both EBLK halves share the stationary
                            # oc slice; one fat tile holds both eb psums
                            pssf = ps2.tile([P, 2, QBLK], F32, name="gen2",
                                            tag="gen2")
                            for j in range(DI_TILES):
                                for eb in range(2):
                                    nc.tensor.matmul(
                                        pssf[:, eb, 0:EBLK], oc[j][:, pcols],
                                        wo_t[j][:, eb * EBLK:(eb + 1) * EBLK],
                                        start=(j == 0),
                                        stop=(j == DI_TILES - 1))
                            fin = fep.tile([P, EMB], MMDT, name="fin",
                                           tag="fin")
                            with nc.allow_low_precision(
                                    reason="fp16 output storage"):
                                nc.vector.tensor_copy(
                                    fin.rearrange("p (a b) -> p a b", b=EBLK),
                                    pssf[:, :, 0:EBLK])
                            nc.gpsimd.dma_start(out=out[scols, :], in_=fin)

                    # software pipeline over steps, merged at kt granularity.
                    # LAG trades o-bank-recycle margin (norm of step s-1 must
                    # complete within LAG slots) against the half-empty
                    # attV-only tail slots each step emits: HW-measured
                    # LAG=3 beats LAG=6 by ~50us.
                    LAG = 3
                    pending_p3 = []
                    scores_prep(0)
                    for kt in range(S_TILES):
                        scores_kt(0, kt)
                    for s in range(nb):
                        has_next = s + 1 < nb
                        if has_next:
                            scores_prep(s + 1)
                        attv_prep(s)
                        nkt = S_TILES + LAG if has_next else S_TILES
                        for kt in range(nkt):
                            if has_next and kt < S_TILES:
                                scores_kt(s + 1, kt)
                            if kt == 0 and s - 1 >= 0:
                                emit_norm(s - 1)
                                qbp_prev, h_prev = blocks[s - 1]
                                if h_prev == HEADS - 1:
                                    pending_p3 += [2 * qbp_prev,
                                                   2 * qbp_prev + 1]
                            akt = kt - LAG if has_next else kt
                            if 0 <= akt < S_TILES:
                                attv_kt(s, akt)
                        attv_fin(s)
                        for qb in pending_p3:
                            emit_phase3(qb)
                        pending_p3 = []
                    emit_norm(nb - 1)
                    emit_phase3(N_QBLK - 2)
                    emit_phase3(N_QBLK - 1)

    nc.compile()
    return nc


_CACHE = {}


def _get_runner(reps=1, phases=3):
    """Build nc once and a reusable jitted SPMD callable (no recompiles)."""
    key = f"runner{reps}_{phases}"
    if key in _CACHE:
        return _CACHE[key]

    import jax
    import numpy as _np
    from jax.sharding import Mesh, PartitionSpec
    from jax.experimental.shard_map import shard_map
    from concourse import bass2jax
    from concourse.bass2jax import _bass_exec_p, install_neuronx_cc_hook

    nc = _build_nc(reps, phases)
    install_neuronx_cc_hook()

    partition_name = (nc.partition_id_tensor.name
                      if nc.partition_id_tensor else None)
    in_names, out_names, out_avals, zero_outs = [], [], [], []
    for alloc in nc.m.functions[0].allocations:
        if not isinstance(alloc, mybir.MemoryLocationSet):
            continue
        name = alloc.memorylocations[0].name
        if alloc.kind == "ExternalInput":
            if name != partition_name:
                in_names.append(name)
        elif alloc.kind == "ExternalOutput":
            shape = list(alloc.tensor_shape)
            npdt = mybir.dt.np(alloc.dtype)
            out_avals.append(jax.core.ShapedArray(shape, npdt))
            out_names.append(name)
            zero_outs.append(_np.zeros(shape, npdt))
    n_params = len(in_names)
    n_outs = len(out_names)
    in_names = in_names + out_names
    if partition_name is not None:
        in_names.append(partition_name)

    def _body(*args):
        operands = list(args)
        if partition_name is not None:
            operands.append(bass2jax.partition_id_tensor())
        outs = _bass_exec_p.bind(
            *operands,
            out_avals=tuple(out_avals),
            in_names=tuple(in_names),
            out_names=tuple(out_names),
            lowering_input_output_aliases=(),
            sim_require_finite=True,
            sim_require_nnan=True,
            nc=nc,
        )
        return tuple(outs)

    devices = jax.devices()[:NCORES]
    mesh = Mesh(_np.asarray(devices), ("core",))
    in_specs = (PartitionSpec("core"),) * (n_params + n_outs)
    out_specs = (PartitionSpec("core"),) * n_outs
    sharded = jax.jit(
        shard_map(_body, mesh=mesh, in_specs=in_specs, out_specs=out_specs,
                  check_rep=False),
        keep_unused=True,
    )
    concat_zeros = [
        _np.zeros((NCORES * z.shape[0], *z.shape[1:]), z.dtype)
        for z in zero_outs
    ]

    runner = {
        "nc": nc, "sharded": sharded, "in_names": in_names,
        "n_params": n_params, "out_names": out_names,
        "out_avals": out_avals, "concat_zeros": concat_zeros,
        "mesh": mesh,
    }
    _CACHE[key] = runner
    return runner


def run_spmd(in_maps):
    """Run the compiled SPMD program; in_maps is a list of NCORES dicts."""
    import numpy as _np
    r = _get_runner()
    per_core = [[_np.asarray(m[name]) for name in r["in_names"][:r["n_params"]]]
                for m in in_maps]
    concat_in = [
        _np.concatenate([per_core[c][i] for c in range(NCORES)], axis=0)
        for i in range(r["n_params"])
    ]
    out_arrs = r["sharded"](*concat_in, *r["concat_zeros"])
    return [
        {name: _np.asarray(out_arrs[i]).reshape(NCORES, *r["out_avals"][i].shape)[c]
         for i, name in enumerate(r["out_names"])}
        for c in range(NCORES)
    ]


def _prep_in_maps(q, k, v, Wq, bq, Wk, bk, Wv, bv, Wo, bo):
    mdt = _np_mmdt()
    q = np.asarray(q, dtype=np.float32)
    k = np.asarray(k, dtype=np.float32)
    v = np.asarray(v, dtype=np.float32)
    wqT = np.ascontiguousarray(np.asarray(Wq, np.float32).T).astype(mdt)
    wkT = np.ascontiguousarray(np.asarray(Wk, np.float32).T).astype(mdt)
    wvT = np.ascontiguousarray(np.asarray(Wv, np.float32).T).astype(mdt)
    woT = np.ascontiguousarray(np.asarray(Wo, np.float32).T).astype(mdt)
    bqc = np.ascontiguousarray(np.asarray(bq, np.float32).reshape(EMB, 1))
    bkc = np.ascontiguousarray(np.asarray(bk, np.float32).reshape(EMB, 1))
    in_maps = []
    for b in range(NCORES):
        in_maps.append({
            "xq": np.ascontiguousarray(q[b].T).astype(mdt),
            "xk": np.ascontiguousarray(k[b].T).astype(mdt),
            "xv": np.ascontiguousarray(v[b].T).astype(mdt),
            "wq": wqT, "wk": wkT, "wv": wvT, "wo": woT,
            "bq": bqc, "bk": bkc,
        })
    return in_maps


def kernel(q, k, v, Wq, bq, Wk, bk, Wv, bv, Wo, bo):
    in_maps = _prep_in_maps(q, k, v, Wq, bq, Wk, bk, Wv, bv, Wo, bo)
    results = run_spmd(in_maps)
    out = np.stack([results[b]["out"] for b in range(NCORES)], axis=0)
    out = out.astype(np.float32)
    # exact epilogue: softmax rows sum to 1, so the V bias contributes
    # bv @ Wo.T to every output row; fold it with bo on the host.
    extra = (np.asarray(bo, np.float32)
             + np.asarray(Wo, np.float32) @ np.asarray(bv, np.float32))
    if np.any(extra):
        out = out + extra
    return out
